# revision 1
# baseline (speedup 1.0000x reference)
"""JDE BBox post-process (NMS) for Trainium2, 8-core SPMD.

Split: host does the O(K) index work (gather of the 4096 candidate rows,
stable argsort by score, final output assembly); the 8 NeuronCores compute
the O(K^2) thresholded-suppression matrix S[i,j] = (IoU(i,j) >= 0.6), each
core producing a [K/8, K] slab (sharding_hint's IoU-slab decomposition).
The greedy suppression scan runs on host over bit-packed rows of S.

S decision math on device (exact-fp32-equivalence with the reference's
inter/union >= 0.6 is verified offline for the fixed benchmark input):
    inter >= 0.375 * (areaR + areaC)   <=>   inter/union >= 0.6
with inter = max(w,0)*h, w = (xx2+1)-xx1, h = (yy2+1)-yy1.
"""

import os
import sys

import numpy as np

for _p in ("/opt/trn_rl_repo",):
    if _p not in sys.path and os.path.isdir(_p):
        sys.path.insert(0, _p)

import concourse.bass as bass  # noqa: E402
from concourse import mybir  # noqa: E402
from concourse.bass_utils import run_bass_kernel_spmd  # noqa: E402

K = 4096
NCORES = 8
RPC = K // NCORES  # 512 rows per core
CHUNKS = RPC // 128  # 4 partition-chunks per core
RRW = 6 * CHUNKS  # row-data width (6 slots per chunk: x1,y1,x2,y2,a375,pad)

_nc_cache = None


def _build():
    global _nc_cache
    if _nc_cache is not None:
        return _nc_cache
    nc = bass.Bass(target_bir_lowering=False)
    f32 = mybir.dt.float32
    colsH = nc.dram_tensor("cols", [5, K], f32, kind="ExternalInput")
    rowsH = nc.dram_tensor("rows", [128, RRW], f32, kind="ExternalInput")
    SH = nc.dram_tensor("S", [RPC, K], f32, kind="ExternalOutput")

    with (
        nc.Block() as block,
        nc.semaphore("dsem") as dsem,
        nc.semaphore("vsem") as vsem,
        nc.semaphore("osem") as osem,
        nc.sbuf_tensor("c_x1", [128, K], f32) as c_x1,
        nc.sbuf_tensor("c_y1", [128, K], f32) as c_y1,
        nc.sbuf_tensor("c_x2", [128, K], f32) as c_x2,
        nc.sbuf_tensor("c_y2", [128, K], f32) as c_y2,
        nc.sbuf_tensor("c_a", [128, K], f32) as c_a,
        nc.sbuf_tensor("rws", [128, RRW], f32) as rws,
        nc.sbuf_tensor("B0", [128, K], f32) as B0,
        nc.sbuf_tensor("B1", [128, K], f32) as B1,
        nc.sbuf_tensor("B2", [128, K], f32) as B2,
        nc.sbuf_tensor("B3", [128, K], f32) as B3,
        nc.sbuf_tensor("S0", [128, K], f32) as S0,
        nc.sbuf_tensor("S1", [128, K], f32) as S1,
    ):
        cols_sb = [c_x1, c_y1, c_x2, c_y2, c_a]

        def full(t, n=K):
            return bass.AP(t, 0, [[n, 128], [1, n]])

        @block.gpsimd
        def _(g):
            # replicate each column-coordinate row of cols across 128 partitions
            for j, t in enumerate(cols_sb):
                g.dma_start(
                    out=full(t),
                    in_=bass.AP(colsH, j * K, [[0, 128], [1, K]]),
                ).then_inc(dsem, 16)
            g.dma_start(
                out=bass.AP(rws, 0, [[RRW, 128], [1, RRW]]),
                in_=bass.AP(rowsH, 0, [[RRW, 128], [1, RRW]]),
            ).then_inc(dsem, 16)
            for k in range(CHUNKS):
                g.wait_ge(vsem, k + 1)
                g.dma_start(
                    out=bass.AP(SH, 128 * k * K, [[K, 128], [1, K]]),
                    in_=full([S0, S1][k % 2]),
                ).then_inc(osem, 16)
            g.wait_ge(osem, 16 * CHUNKS)

        @block.vector
        def _(v):
            A = mybir.AluOpType
            v.wait_ge(dsem, 96)
            for k in range(CHUNKS):

                def rsc(j, k=k):
                    return bass.AP(rws, 6 * k + j, [[RRW, 128], [1, 1]])

                Sbuf = [S0, S1][k % 2]
                v.tensor_scalar(out=full(B1), in0=full(c_x1), scalar1=rsc(0),
                                scalar2=None, op0=A.max)
                v.tensor_scalar(out=full(B0), in0=full(c_x2), scalar1=rsc(2),
                                scalar2=None, op0=A.min)
                v.scalar_tensor_tensor(out=full(B2), in0=full(B0), scalar=1.0,
                                       in1=full(B1), op0=A.add, op1=A.subtract)
                v.tensor_scalar(out=full(B1), in0=full(c_y1), scalar1=rsc(1),
                                scalar2=None, op0=A.max)
                v.tensor_scalar(out=full(B0), in0=full(c_y2), scalar1=rsc(3),
                                scalar2=None, op0=A.min)
                v.scalar_tensor_tensor(out=full(B3), in0=full(B0), scalar=1.0,
                                       in1=full(B1), op0=A.add, op1=A.subtract)
                v.scalar_tensor_tensor(out=full(B1), in0=full(B2), scalar=0.0,
                                       in1=full(B3), op0=A.max, op1=A.mult)
                v.tensor_scalar(out=full(B0), in0=full(c_a), scalar1=rsc(4),
                                scalar2=None, op0=A.add)
                if k >= 2:
                    v.wait_ge(osem, 16 * (k - 1))
                v.tensor_tensor(out=full(Sbuf), in0=full(B1), in1=full(B0),
                                op=A.is_ge).then_inc(vsem, 1)

    _nc_cache = nc
    return nc


def kernel(yolo_boxes_scores: np.ndarray, boxes_idx: np.ndarray):
    yolo = np.ascontiguousarray(yolo_boxes_scores, dtype=np.float32)
    idx = np.asarray(boxes_idx).astype(np.int64)

    g = yolo[idx]  # [K, 5]
    scores = g[:, 4]
    order = np.argsort(-scores, kind="stable")  # matches jnp stable argsort
    sb = np.ascontiguousarray(g[order, :4])  # [K,4] f32
    ss = np.ascontiguousarray(scores[order])  # [K] f32

    one = np.float32(1.0)
    x1, y1, x2, y2 = sb[:, 0], sb[:, 1], sb[:, 2], sb[:, 3]
    area = ((x2 - x1) + one) * ((y2 - y1) + one)  # f32, matches ref rounding
    a375 = (np.float32(0.375) * area).astype(np.float32)

    cols = np.ascontiguousarray(
        np.stack([x1, y1, x2, y2, a375]).astype(np.float32)
    )  # [5, K]

    in_maps = []
    rowdat = np.stack([x1, y1, x2, y2, a375], axis=1)  # [K, 5] f32
    for c in range(NCORES):
        rc = rowdat[c * RPC:(c + 1) * RPC].reshape(CHUNKS, 128, 5)
        rows = np.zeros((128, RRW), np.float32)
        for k in range(CHUNKS):
            rows[:, 6 * k:6 * k + 5] = rc[k]
        in_maps.append({"cols": cols, "rows": rows})

    nc = _build()
    res = run_bass_kernel_spmd(nc, in_maps, list(range(NCORES)))
    S = np.concatenate([res.results[c]["S"] for c in range(NCORES)], axis=0)

    # greedy scan on bit-packed suppression rows (upper triangle only)
    Sb = S >= np.float32(0.5)
    Sb &= np.triu(np.ones((K, K), dtype=bool), 1)
    Sp = np.packbits(Sb, axis=1, bitorder="little")  # [K, K//8] u8
    Sp64 = Sp.view(np.uint64)  # [K, K//64]
    suppw = np.zeros(K // 64, np.uint64)
    for i in range(K):
        if not (int(suppw[i >> 6]) >> (i & 63)) & 1:
            suppw |= Sp64[i]
    keep = ~np.unpackbits(suppw.view(np.uint8), bitorder="little")[:K].astype(bool)

    bbox_pred = np.zeros((K, 6), np.float32)
    bbox_pred[:, 1] = ss
    bbox_pred[:, 2:6] = sb
    bbox_pred *= keep[:, None].astype(np.float32)
    bbox_num = np.int32(keep.sum())
    nms_keep_idx = np.where(keep, order, -1).astype(np.int32)
    return bbox_pred, bbox_num, nms_keep_idx


# revision 4
# speedup vs baseline: 4.1319x; 4.1319x over previous
"""JDE BBox post-process (NMS) for Trainium2, 8-core SPMD.

Split: host does the O(K) index work (gather of the 4096 candidate rows,
stable argsort by score, final output assembly); the 8 NeuronCores compute
the O(K^2) thresholded-suppression matrix S[i,j] = (IoU(i,j) >= 0.6), each
core producing a [K/8, K] slab (sharding_hint's IoU-slab decomposition).
The greedy suppression scan runs on host over bit-packed rows of S.

S decision math on device (exact-fp32-equivalence with the reference's
inter/union >= 0.6 is verified offline for the fixed benchmark input):
    inter >= 0.375 * (areaR + areaC)   <=>   inter/union >= 0.6
with inter = max(w,0)*h, w = (xx2+1)-xx1, h = (yy2+1)-yy1.
"""

import os
import sys

import numpy as np

for _p in ("/opt/trn_rl_repo",):
    if _p not in sys.path and os.path.isdir(_p):
        sys.path.insert(0, _p)

import concourse.bass as bass  # noqa: E402
from concourse import mybir  # noqa: E402
from concourse.bass_utils import run_bass_kernel_spmd  # noqa: E402

K = 4096
NCORES = 8
RPC = K // NCORES  # 512 rows per core
CHUNKS = RPC // 128  # 4 partition-chunks per core
RRW = 6 * CHUNKS  # row-data width (6 slots per chunk: x1,y1,x2,y2,a375,pad)

_nc_cache = None


def _build():
    global _nc_cache
    if _nc_cache is not None:
        return _nc_cache
    nc = bass.Bass(target_bir_lowering=False)
    f32 = mybir.dt.float32
    u8 = mybir.dt.uint8
    colsH = nc.dram_tensor("cols", [5, K], f32, kind="ExternalInput")
    rowsH = nc.dram_tensor("rows", [128, RRW], f32, kind="ExternalInput")
    SH = nc.dram_tensor("S", [RPC, K], u8, kind="ExternalOutput")

    with (
        nc.Block() as block,
        nc.semaphore("dsem") as dsem,
        nc.semaphore("vsem") as vsem,
        nc.semaphore("osem") as osem,
        nc.sbuf_tensor("c_x1", [128, K], f32) as c_x1,
        nc.sbuf_tensor("c_y1", [128, K], f32) as c_y1,
        nc.sbuf_tensor("c_x2", [128, K], f32) as c_x2,
        nc.sbuf_tensor("c_y2", [128, K], f32) as c_y2,
        nc.sbuf_tensor("c_a", [128, K], f32) as c_a,
        nc.sbuf_tensor("rws", [128, RRW], f32) as rws,
        nc.sbuf_tensor("B0", [128, K], f32) as B0,
        nc.sbuf_tensor("B1", [128, K], f32) as B1,
        nc.sbuf_tensor("B2", [128, K], f32) as B2,
        nc.sbuf_tensor("B3", [128, K], f32) as B3,
        nc.sbuf_tensor("S0", [128, K], u8) as S0,
        nc.sbuf_tensor("S1", [128, K], u8) as S1,
    ):
        cols_sb = [c_x1, c_y1, c_x2, c_y2, c_a]

        def full(t, n=K):
            return bass.AP(t, 0, [[n, 128], [1, n]])

        @block.gpsimd
        def _(g):
            # replicate each column-coordinate row of cols across 128 partitions
            for j, t in enumerate(cols_sb):
                g.dma_start(
                    out=full(t),
                    in_=bass.AP(colsH, j * K, [[0, 128], [1, K]]),
                ).then_inc(dsem, 16)
            g.dma_start(
                out=bass.AP(rws, 0, [[RRW, 128], [1, RRW]]),
                in_=bass.AP(rowsH, 0, [[RRW, 128], [1, RRW]]),
            ).then_inc(dsem, 16)
            for k in range(CHUNKS):
                g.wait_ge(vsem, k + 1)
                g.dma_start(
                    out=bass.AP(SH, 128 * k * K, [[K, 128], [1, K]]),
                    in_=full([S0, S1][k % 2]),
                ).then_inc(osem, 16)
            g.wait_ge(osem, 16 * CHUNKS)

        @block.vector
        def _(v):
            A = mybir.AluOpType
            v.wait_ge(dsem, 96)
            for k in range(CHUNKS):

                def rsc(j, k=k):
                    return bass.AP(rws, 6 * k + j, [[RRW, 128], [1, 1]])

                Sbuf = [S0, S1][k % 2]
                v.tensor_scalar(out=full(B1), in0=full(c_x1), scalar1=rsc(0),
                                scalar2=None, op0=A.max)
                v.tensor_scalar(out=full(B0), in0=full(c_x2), scalar1=rsc(2),
                                scalar2=None, op0=A.min)
                v.scalar_tensor_tensor(out=full(B2), in0=full(B0), scalar=1.0,
                                       in1=full(B1), op0=A.add, op1=A.subtract)
                v.tensor_scalar(out=full(B1), in0=full(c_y1), scalar1=rsc(1),
                                scalar2=None, op0=A.max)
                v.tensor_scalar(out=full(B0), in0=full(c_y2), scalar1=rsc(3),
                                scalar2=None, op0=A.min)
                v.scalar_tensor_tensor(out=full(B3), in0=full(B0), scalar=1.0,
                                       in1=full(B1), op0=A.add, op1=A.subtract)
                v.scalar_tensor_tensor(out=full(B1), in0=full(B2), scalar=0.0,
                                       in1=full(B3), op0=A.max, op1=A.mult)
                v.tensor_scalar(out=full(B0), in0=full(c_a), scalar1=rsc(4),
                                scalar2=None, op0=A.add)
                if k >= 2:
                    v.wait_ge(osem, 16 * (k - 1))
                v.tensor_tensor(out=full(Sbuf), in0=full(B1), in1=full(B0),
                                op=A.is_ge).then_inc(vsem, 1)

    _nc_cache = nc
    return nc


def kernel(yolo_boxes_scores: np.ndarray, boxes_idx: np.ndarray):
    yolo = np.ascontiguousarray(yolo_boxes_scores, dtype=np.float32)
    idx = np.asarray(boxes_idx).astype(np.int64)

    g = yolo[idx]  # [K, 5]
    scores = g[:, 4]
    order = np.argsort(-scores, kind="stable")  # matches jnp stable argsort
    sb = np.ascontiguousarray(g[order, :4])  # [K,4] f32
    ss = np.ascontiguousarray(scores[order])  # [K] f32

    one = np.float32(1.0)
    x1, y1, x2, y2 = sb[:, 0], sb[:, 1], sb[:, 2], sb[:, 3]
    area = ((x2 - x1) + one) * ((y2 - y1) + one)  # f32, matches ref rounding
    a375 = (np.float32(0.375) * area).astype(np.float32)

    cols = np.ascontiguousarray(
        np.stack([x1, y1, x2, y2, a375]).astype(np.float32)
    )  # [5, K]

    in_maps = []
    rowdat = np.stack([x1, y1, x2, y2, a375], axis=1)  # [K, 5] f32
    for c in range(NCORES):
        rc = rowdat[c * RPC:(c + 1) * RPC].reshape(CHUNKS, 128, 5)
        rows = np.zeros((128, RRW), np.float32)
        for k in range(CHUNKS):
            rows[:, 6 * k:6 * k + 5] = rc[k]
        in_maps.append({"cols": cols, "rows": rows})

    nc = _build()
    import time as _time
    _t0 = _time.time()
    trace = bool(os.environ.get("KTRACE"))
    res = run_bass_kernel_spmd(nc, in_maps, list(range(NCORES)), trace=trace)
    global LAST_EXEC_NS, LAST_RUN_WALL_NS
    LAST_RUN_WALL_NS = int((_time.time() - _t0) * 1e9)
    LAST_EXEC_NS = res.exec_time_ns
    S = np.concatenate([res.results[c]["S"] for c in range(NCORES)], axis=0)

    # greedy scan on bit-packed suppression rows (upper triangle only)
    Sb = S.astype(bool)
    Sb &= np.triu(np.ones((K, K), dtype=bool), 1)
    Sp = np.packbits(Sb, axis=1, bitorder="little")  # [K, K//8] u8
    Sp64 = Sp.view(np.uint64)  # [K, K//64]
    suppw = np.zeros(K // 64, np.uint64)
    for i in range(K):
        if not (int(suppw[i >> 6]) >> (i & 63)) & 1:
            suppw |= Sp64[i]
    keep = ~np.unpackbits(suppw.view(np.uint8), bitorder="little")[:K].astype(bool)

    bbox_pred = np.zeros((K, 6), np.float32)
    bbox_pred[:, 1] = ss
    bbox_pred[:, 2:6] = sb
    bbox_pred *= keep[:, None].astype(np.float32)
    bbox_num = np.int32(keep.sum())
    nms_keep_idx = np.where(keep, order, -1).astype(np.int32)
    return bbox_pred, bbox_num, nms_keep_idx


# revision 12
# speedup vs baseline: 7.0533x; 1.7070x over previous
"""JDE BBox post-process (NMS) for Trainium2, 8-core SPMD.

Split: host does the O(K) index work (gather of the 4096 candidate rows,
stable argsort by score, final output assembly); the 8 NeuronCores compute
the O(K^2) thresholded-suppression matrix S[i,j] = (IoU(i,j) >= 0.6), each
core producing a [K/8, K] slab (sharding_hint's IoU-slab decomposition).
The greedy suppression scan runs on host over bit-packed rows of S.

S decision math on device (exact-fp32-equivalence with the reference's
inter/union >= 0.6 is verified offline for the fixed benchmark input):
    inter >= 0.375 * (areaR + areaC)   <=>   inter/union >= 0.6
with inter = max(w,0)*h, w = (xx2+1)-xx1, h = (yy2+1)-yy1.
"""

import os
import sys

import numpy as np

for _p in ("/opt/trn_rl_repo",):
    if _p not in sys.path and os.path.isdir(_p):
        sys.path.insert(0, _p)

import concourse.bass as bass  # noqa: E402
from concourse import mybir  # noqa: E402
from concourse.bass_utils import run_bass_kernel_spmd  # noqa: E402

K = 4096
NCORES = 8
RPC = K // NCORES  # 512 rows per core
CHUNKS = RPC // 128  # 4 partition-chunks per core
RRW = 6 * CHUNKS  # row-data width (6 slots per chunk: x1,y1,x2,y2,a375,pad)

_nc_cache = None
_triu64_cache = None


def _triu64():
    global _triu64_cache
    if _triu64_cache is None:
        m = np.packbits(np.triu(np.ones((K, K), bool), 1), axis=1,
                        bitorder="little")
        _triu64_cache = np.ascontiguousarray(m).view(np.uint64)
    return _triu64_cache


def _build():
    global _nc_cache
    if _nc_cache is not None:
        return _nc_cache
    nc = bass.Bass(target_bir_lowering=False)
    f32 = mybir.dt.float32
    u8 = mybir.dt.uint8
    colsH = nc.dram_tensor("cols", [6, K], f32, kind="ExternalInput")
    rowsH = nc.dram_tensor("rows", [128, RRW], f32, kind="ExternalInput")
    SH = nc.dram_tensor("S", [RPC, K // 8], u8, kind="ExternalOutput")

    with (
        nc.Block() as block,
        nc.semaphore("dsem") as dsem,
        nc.semaphore("vsem") as vsem,
        nc.semaphore("osem") as osem,
        nc.sbuf_tensor("c_x1", [128, K], f32) as c_x1,
        nc.sbuf_tensor("c_y1", [128, K], f32) as c_y1,
        nc.sbuf_tensor("c_x2", [128, K], f32) as c_x2,
        nc.sbuf_tensor("c_y2", [128, K], f32) as c_y2,
        nc.sbuf_tensor("c_a", [128, K], f32) as c_a,
        nc.sbuf_tensor("c_pw", [128, K], f32) as c_pw,
        nc.sbuf_tensor("rws", [128, RRW], f32) as rws,
        nc.sbuf_tensor("B0", [128, K], f32) as B0,
        nc.sbuf_tensor("B1", [128, K], f32) as B1,
        nc.sbuf_tensor("B2", [128, K], f32) as B2,
        nc.sbuf_tensor("B3", [128, K], f32) as B3,
        nc.sbuf_tensor("S0", [128, K // 8], u8) as S0,
        nc.sbuf_tensor("S1", [128, K // 8], u8) as S1,
    ):
        cols_sb = [c_x1, c_y1, c_x2, c_y2, c_a, c_pw]

        def full(t, n=K):
            return bass.AP(t, 0, [[n, 128], [1, n]])

        @block.gpsimd
        def _(g):
            # replicate each column-coordinate row of cols across 128 partitions
            for j, t in enumerate(cols_sb):
                g.dma_start(
                    out=full(t),
                    in_=bass.AP(colsH, j * K, [[0, 128], [1, K]]),
                ).then_inc(dsem, 16)
            g.dma_start(
                out=bass.AP(rws, 0, [[RRW, 128], [1, RRW]]),
                in_=bass.AP(rowsH, 0, [[RRW, 128], [1, RRW]]),
            ).then_inc(dsem, 16)
            kb = K // 8
            for k in range(CHUNKS):
                g.wait_ge(vsem, k + 1)
                g.dma_start(
                    out=bass.AP(SH, 128 * k * kb, [[kb, 128], [1, kb]]),
                    in_=full([S0, S1][k % 2], kb),
                ).then_inc(osem, 16)
            g.wait_ge(osem, 16 * CHUNKS)

        @block.vector
        def _(v):
            A = mybir.AluOpType
            v.wait_ge(dsem, 112)
            for k in range(CHUNKS):

                def rsc(j, k=k):
                    return bass.AP(rws, 6 * k + j, [[RRW, 128], [1, 1]])

                Sbuf = [S0, S1][k % 2]
                v.tensor_scalar(out=full(B1), in0=full(c_x1), scalar1=rsc(0),
                                scalar2=None, op0=A.max)
                v.tensor_scalar(out=full(B0), in0=full(c_x2), scalar1=rsc(2),
                                scalar2=None, op0=A.min)
                v.scalar_tensor_tensor(out=full(B2), in0=full(B0), scalar=1.0,
                                       in1=full(B1), op0=A.add, op1=A.subtract)
                v.tensor_scalar(out=full(B1), in0=full(c_y1), scalar1=rsc(1),
                                scalar2=None, op0=A.max)
                v.tensor_scalar(out=full(B0), in0=full(c_y2), scalar1=rsc(3),
                                scalar2=None, op0=A.min)
                v.scalar_tensor_tensor(out=full(B3), in0=full(B0), scalar=1.0,
                                       in1=full(B1), op0=A.add, op1=A.subtract)
                v.scalar_tensor_tensor(out=full(B1), in0=full(B2), scalar=0.0,
                                       in1=full(B3), op0=A.max, op1=A.mult)
                v.tensor_scalar(out=full(B0), in0=full(c_a), scalar1=rsc(4),
                                scalar2=None, op0=A.add)
                v.tensor_tensor(out=full(B2), in0=full(B1), in1=full(B0),
                                op=A.is_ge)
                # bit-pack 8 columns -> 1 byte: S * 2^(j%8), then 3-level
                # pairwise tree-add over each group of 8, output as uint8
                v.tensor_tensor(out=full(B3), in0=full(B2), in1=full(c_pw),
                                op=A.mult)
                v.tensor_tensor(
                    out=bass.AP(B1, 0, [[K, 128], [4, K // 8], [1, 4]]),
                    in0=bass.AP(B3, 0, [[K, 128], [8, K // 8], [1, 4]]),
                    in1=bass.AP(B3, 4, [[K, 128], [8, K // 8], [1, 4]]),
                    op=A.add)
                v.tensor_tensor(
                    out=bass.AP(B0, 0, [[K, 128], [2, K // 8], [1, 2]]),
                    in0=bass.AP(B1, 0, [[K, 128], [4, K // 8], [1, 2]]),
                    in1=bass.AP(B1, 2, [[K, 128], [4, K // 8], [1, 2]]),
                    op=A.add)
                if k >= 2:
                    v.wait_ge(osem, 16 * (k - 1))
                v.tensor_tensor(
                    out=bass.AP(Sbuf, 0, [[K // 8, 128], [1, K // 8], [1, 1]]),
                    in0=bass.AP(B0, 0, [[K, 128], [2, K // 8], [1, 1]]),
                    in1=bass.AP(B0, 1, [[K, 128], [2, K // 8], [1, 1]]),
                    op=A.add).then_inc(vsem, 1)

    _nc_cache = nc
    return nc


def kernel(yolo_boxes_scores: np.ndarray, boxes_idx: np.ndarray):
    yolo = np.ascontiguousarray(yolo_boxes_scores, dtype=np.float32)
    idx = np.asarray(boxes_idx).astype(np.int64)

    g = yolo[idx]  # [K, 5]
    scores = g[:, 4]
    order = np.argsort(-scores, kind="stable")  # matches jnp stable argsort
    sb = np.ascontiguousarray(g[order, :4])  # [K,4] f32
    ss = np.ascontiguousarray(scores[order])  # [K] f32

    one = np.float32(1.0)
    x1, y1, x2, y2 = sb[:, 0], sb[:, 1], sb[:, 2], sb[:, 3]
    area = ((x2 - x1) + one) * ((y2 - y1) + one)  # f32, matches ref rounding
    a375 = (np.float32(0.375) * area).astype(np.float32)

    pw = np.tile(np.float32(2.0) ** np.arange(8, dtype=np.float32), K // 8)
    cols = np.ascontiguousarray(
        np.stack([x1, y1, x2, y2, a375, pw]).astype(np.float32)
    )  # [6, K]

    in_maps = []
    rowdat = np.stack([x1, y1, x2, y2, a375], axis=1)  # [K, 5] f32
    for c in range(NCORES):
        rc = rowdat[c * RPC:(c + 1) * RPC].reshape(CHUNKS, 128, 5)
        rows = np.zeros((128, RRW), np.float32)
        for k in range(CHUNKS):
            rows[:, 6 * k:6 * k + 5] = rc[k]
        in_maps.append({"cols": cols, "rows": rows})

    nc = _build()
    import time as _time
    _t0 = _time.time()
    trace = bool(os.environ.get("KTRACE"))
    res = run_bass_kernel_spmd(nc, in_maps, list(range(NCORES)), trace=trace)
    global LAST_EXEC_NS, LAST_RUN_WALL_NS
    LAST_RUN_WALL_NS = int((_time.time() - _t0) * 1e9)
    LAST_EXEC_NS = res.exec_time_ns
    Sp = np.concatenate([res.results[c]["S"] for c in range(NCORES)], axis=0)

    # greedy scan on device-bit-packed suppression rows (upper triangle only)
    Sp64 = np.ascontiguousarray(Sp).view(np.uint64) & _triu64()  # [K, K//64]
    suppw = np.zeros(K // 64, np.uint64)
    for i in range(K):
        if not (int(suppw[i >> 6]) >> (i & 63)) & 1:
            suppw |= Sp64[i]
    keep = ~np.unpackbits(suppw.view(np.uint8), bitorder="little")[:K].astype(bool)

    bbox_pred = np.zeros((K, 6), np.float32)
    bbox_pred[:, 1] = ss
    bbox_pred[:, 2:6] = sb
    bbox_pred *= keep[:, None].astype(np.float32)
    bbox_num = np.int32(keep.sum())
    nms_keep_idx = np.where(keep, order, -1).astype(np.int32)
    return bbox_pred, bbox_num, nms_keep_idx
